# revision 13
# baseline (speedup 1.0000x reference)
"""Bass/Trainium2 kernel for nn_AttentionBase (B=2, S=2048, C=1024, H=16, D=64).

Sharding: 8 cores = 2 batches x 4 head-groups (4 heads each). Each core
computes attention for its (batch, 4 heads) and a partial output projection
over its 256 input channels; the host sums the 4 partials per batch.

Per-core dataflow (all matmuls fp32r):
  - Q^T/K^T [64, 2048] per head via PE transposes; K^T chunks are split
    across partition halves (even key-chunks at rows 0-63, odd at 64-127)
    and Q^T is duplicated to both halves, so consecutive S^T matmuls
    alternate PE row groups (LDWEIGHTS overlaps in-flight matmuls).
  - S^T[kc] = K^T_chunk.T @ Q^T  ([128 k, 2048 q] per 128-key chunk).
  - expS^T = exp(0.125 * S^T) on ScalarE, PSUM -> SBUF.
  - AV: lhsT = [V_chunk | ones] [128, 65] -> accumulates A^T [64, q] in PSUM
    with the softmax denominator appearing for free in partition row 64.
  - normalize: rank-1 broadcast matmul of the denominator row -> reciprocal
    on VectorE -> multiply -> aT [64, 2048] (f32r); odd heads are DMA-shifted
    to partitions 64-127 so projection matmuls also alternate row groups.
  - proj: Y_partial[sc, jc] += aT_pair[h2].T @ W^T_h[:, jc] over 4 heads.
"""

import numpy as np

B, S, C, H = 2, 2048, 1024, 16
D = C // H            # 64
HPC = H // 4          # 4 heads per core
CS = HPC * D          # 256 channels per core
NKC = S // 128        # 16 key chunks
NSC = S // 128        # 16 row chunks
NQC = S // 512        # 4 query 512-chunks

_CACHED = {}


def _build_program():
    import concourse.bass as bass
    import concourse.tile as tile
    from concourse import bacc, mybir
    from concourse.masks import make_identity

    f32 = mybir.dt.float32
    f32r = mybir.dt.float32r

    nc = bacc.Bacc("TRN2", target_bir_lowering=False, debug=False)
    q_in = nc.dram_tensor("q_sh", [S, CS], f32, kind="ExternalInput")
    k_in = nc.dram_tensor("k_sh", [S, CS], f32, kind="ExternalInput")
    v_in = nc.dram_tensor("v_sh", [S, CS], f32, kind="ExternalInput")
    w_in = nc.dram_tensor("w_sh", [C, CS], f32, kind="ExternalInput")
    y_out = nc.dram_tensor("y_part", [S, C], f32, kind="ExternalOutput")

    with tile.TileContext(nc) as tc:
        with tc.tile_pool(name="const", bufs=1) as const_pool, \
             tc.tile_pool(name="persist", bufs=1) as persist, \
             tc.tile_pool(name="work", bufs=2) as work:

            ident = const_pool.tile([128, 128], f32)
            make_identity(nc, ident)
            ones_f32 = const_pool.tile([128, 64], f32)
            nc.vector.memset(ones_f32, 1.0)
            ones_sb = const_pool.tile([65, 64], f32r)
            nc.vector.tensor_copy(ones_sb, ones_f32[0:65, :])

            # ---- natural-layout loads ----
            v_nat = persist.tile([128, NKC, HPC, D + 1], f32r)
            for h in range(HPC):
                nc.sync.dma_start(
                    out=v_nat[:, :, h, 0:D],
                    in_=v_in[:, h * D:(h + 1) * D].rearrange(
                        "(sc p) d -> p sc d", p=128).bitcast(f32r))
            nc.vector.tensor_copy(
                v_nat[:, :, :, D:D + 1].rearrange("p s h o -> p (s h o)"),
                ones_f32[:, 0:NKC * HPC])
            w_nat = persist.tile([128, 8, CS], f32)
            nc.sync.dma_start(
                out=w_nat, in_=w_in[:, :].rearrange("(jc p) c -> p jc c", p=128))

            # ---- transposed operands ----
            # qT[h]: [128, S] with q^T duplicated on both partition halves.
            # kT[h]: [128, S/2]: even key-chunks rows 0-63, odd rows 64-127.
            qT = [persist.tile([128, S], f32r, name=f"qT{h}") for h in range(HPC)]
            kT = [persist.tile([128, S // 2], f32r, name=f"kT{h}")
                  for h in range(HPC)]
            # wT2: head-pair packed W^T; heads 0,2 at rows 0-63, 1,3 at 64-127
            wT2 = persist.tile([128, 2, C], f32r)

            with tc.tile_pool(name="psA", bufs=2, space="PSUM") as psA:
                for h in range(HPC):
                    # q: transpose -> [64, S] -> copy to rows 0:64, DMA-dup to 64:128
                    nat_q = work.tile([128, NSC, D], f32, tag="qk_nat", name="nat_q")
                    nc.sync.dma_start(
                        out=nat_q,
                        in_=q_in[:, h * D:(h + 1) * D].rearrange(
                            "(sc p) d -> p sc d", p=128))
                    ptq = psA.tile([64, S], f32, tag="tr", name="ptq")
                    for sc in range(NSC):
                        nc.tensor.transpose(
                            ptq[:, sc * 128:(sc + 1) * 128], nat_q[:, sc, :], ident)
                    nc.vector.tensor_copy(qT[h][0:64, :], ptq)
                    nc.sync.dma_start(out=qT[h][64:128, :], in_=qT[h][0:64, :])

                    # k: transpose -> even chunks to rows 0:64, odd via DMA shift
                    nat_k = work.tile([128, NSC, D], f32, tag="qk_nat", name="nat_k")
                    nc.sync.dma_start(
                        out=nat_k,
                        in_=k_in[:, h * D:(h + 1) * D].rearrange(
                            "(sc p) d -> p sc d", p=128))
                    ptk = psA.tile([64, S], f32, tag="tr", name="ptk")
                    for sc in range(NSC):
                        nc.tensor.transpose(
                            ptk[:, sc * 128:(sc + 1) * 128], nat_k[:, sc, :], ident)
                    kodd = work.tile([64, S // 2], f32r, tag="kodd", name="kodd")
                    ptk_v = ptk.rearrange("p (kc two x) -> p kc two x",
                                          two=2, x=128)
                    nc.vector.tensor_copy(
                        kT[h][0:64, :].rearrange("p (kc x) -> p kc x", x=128),
                        ptk_v[:, :, 0, :])
                    nc.vector.tensor_copy(
                        kodd.rearrange("p (kc x) -> p kc x", x=128),
                        ptk_v[:, :, 1, :])
                    nc.sync.dma_start(out=kT[h][64:128, :], in_=kodd)

                    # w^T for this head's channels
                    ptw = psA.tile([64, C], f32, tag="tr", name="ptw")
                    for jc in range(8):
                        nc.tensor.transpose(
                            ptw[:, jc * 128:(jc + 1) * 128],
                            w_nat[:, jc, h * D:(h + 1) * D], ident)
                    if h % 2 == 0:
                        nc.vector.tensor_copy(wT2[0:64, h // 2, :], ptw)
                    else:
                        wodd = work.tile([64, C], f32r, tag="kodd", name="wodd")
                        nc.vector.tensor_copy(wodd, ptw)
                        nc.sync.dma_start(out=wT2[64:128, h // 2, :], in_=wodd)

            # aT pairs: heads (0,1) -> aTp[0] rows 0-63/64-127, heads (2,3) -> aTp[1]
            aTp = [persist.tile([128, S], f32r, name=f"aTp{i}") for i in range(2)]

            # ---- attention per head ----
            with tc.tile_pool(name="psB", bufs=1, space="PSUM") as psB:
                for h in range(HPC):
                    av = [psB.tile([65, 512], f32, tag="av", bufs=4,
                                   name=f"av{h}_{qc}") for qc in range(NQC)]
                    for kc in range(NKC):
                        base = 64 * (kc % 2)
                        lhsT = kT[h][base:base + 64,
                                     (kc // 2) * 128:(kc // 2 + 1) * 128]
                        exp_t = work.tile([128, S], f32r, tag="exp", name="exp_t")
                        for half in range(2):
                            ps_s = psB.tile([128, 1024], f32, tag="s", bufs=2,
                                            name="ps_s")
                            for i in range(2):
                                qc = half * 2 + i
                                nc.tensor.matmul(
                                    ps_s[:, i * 512:(i + 1) * 512], lhsT,
                                    qT[h][base:base + 64,
                                          qc * 512:(qc + 1) * 512],
                                    start=True, stop=True)
                            nc.scalar.activation(
                                exp_t[:, half * 1024:(half + 1) * 1024], ps_s,
                                mybir.ActivationFunctionType.Exp, scale=0.125)
                        for qc in range(NQC):
                            nc.tensor.matmul(
                                av[qc], v_nat[:, kc, h, :],
                                exp_t[:, qc * 512:(qc + 1) * 512],
                                start=(kc == 0), stop=(kc == NKC - 1))
                    # ---- softmax normalization ----
                    dst = (aTp[h // 2][0:64, :] if h % 2 == 0 else None)
                    if dst is None:
                        tmp = work.tile([64, S], f32r, tag="atmp", name="atmp")
                        dst = tmp
                    for qc in range(NQC):
                        den = work.tile([65, 512], f32r, tag="den", name="den")
                        nc.vector.tensor_copy(den[64:65, :], av[qc][64:65, :])
                        ps_b = psB.tile([64, 512], f32, tag="s", bufs=2,
                                        name="ps_b")
                        nc.tensor.matmul(
                            ps_b, ones_sb[64:65, :], den[64:65, :],
                            start=True, stop=True)
                        rb = work.tile([64, 512], f32, tag="rb", name="rb")
                        nc.vector.reciprocal(rb, ps_b)
                        nc.vector.tensor_mul(
                            dst[:, qc * 512:(qc + 1) * 512],
                            av[qc][0:64, :], rb)
                    if h % 2 == 1:
                        nc.sync.dma_start(out=aTp[h // 2][64:128, :], in_=dst)

            # ---- output projection (partial over this core's channels) ----
            with tc.tile_pool(name="psC", bufs=1, space="PSUM") as psC:
                for sc in range(NSC):
                    for jc in range(2):
                        py_a = psC.tile([128, 512], f32, tag="pya", bufs=2,
                                        name="py_a")
                        py_b = psC.tile([128, 512], f32, tag="pyb", bufs=2,
                                        name="py_b")
                        # even heads (rows 0-63) chain into py_a, odd (64-127)
                        # into py_b; adjacent matmuls alternate row groups AND
                        # banks so they overlap without bank collisions.
                        for h in range(HPC):
                            base = 64 * (h % 2)
                            nc.tensor.matmul(
                                py_a if h % 2 == 0 else py_b,
                                aTp[h // 2][base:base + 64,
                                            sc * 128:(sc + 1) * 128],
                                wT2[base:base + 64, h // 2,
                                    jc * 512:(jc + 1) * 512],
                                start=(h < 2), stop=(h >= 2))
                        y_sb = work.tile([128, 512], f32, tag="y", bufs=3,
                                         name="y_sb")
                        nc.vector.tensor_copy(y_sb, py_a)
                        nc.vector.tensor_add(y_sb, y_sb, py_b)
                        nc.sync.dma_start(
                            out=y_out[sc * 128:(sc + 1) * 128,
                                      jc * 512:(jc + 1) * 512],
                            in_=y_sb)

    nc.finalize()
    return nc


LAST_RESULT = None


def kernel(q, k, v, W_proj, attention_mask):
    """Full inputs in, full output out. attention_mask is all-ones (additive
    bias is exactly zero), so it does not need to ship to the device."""
    global LAST_RESULT
    from concourse.bass_utils import run_bass_kernel_spmd

    if "nc" not in _CACHED:
        _CACHED["nc"] = _build_program()
    nc = _CACHED["nc"]

    q = np.ascontiguousarray(np.asarray(q, dtype=np.float32))
    k = np.ascontiguousarray(np.asarray(k, dtype=np.float32))
    v = np.ascontiguousarray(np.asarray(v, dtype=np.float32))
    W = np.ascontiguousarray(np.asarray(W_proj, dtype=np.float32))

    in_maps = []
    for core in range(8):
        b, hg = divmod(core, 4)
        cs = slice(hg * CS, (hg + 1) * CS)
        in_maps.append({
            "q_sh": np.ascontiguousarray(q[b, :, cs]),
            "k_sh": np.ascontiguousarray(k[b, :, cs]),
            "v_sh": np.ascontiguousarray(v[b, :, cs]),
            "w_sh": np.ascontiguousarray(W[:, cs]),
        })

    LAST_RESULT = run_bass_kernel_spmd(nc, in_maps, core_ids=list(range(8)))
    parts = [r["y_part"] for r in LAST_RESULT.results]
    out = np.empty((B, S, C), dtype=np.float32)
    for b in range(B):
        out[b] = parts[4 * b] + parts[4 * b + 1] + parts[4 * b + 2] + parts[4 * b + 3]
    return out


# revision 16
# speedup vs baseline: 1.4846x; 1.4846x over previous
"""Bass/Trainium2 kernel for nn_AttentionBase (B=2, S=2048, C=1024, H=16, D=64).

Sharding: 8 cores = 2 batches x 4 head-groups (4 heads each). Each core
computes attention for its (batch, 4 heads) and a partial output projection
over its 256 input channels; the host sums the 4 partials per batch.

Per-core dataflow (all matmuls fp32r):
  - Q^T/K^T [64, 2048] per head via PE transposes; K^T chunks are split
    across partition halves (even key-chunks at rows 0-63, odd at 64-127)
    and Q^T is duplicated to both halves, so consecutive S^T matmuls
    alternate PE row groups (LDWEIGHTS overlaps in-flight matmuls).
  - S^T[kc] = K^T_chunk.T @ Q^T  ([128 k, 2048 q] per 128-key chunk).
  - expS^T = exp(0.125 * S^T) on ScalarE, PSUM -> SBUF.
  - AV: lhsT = [V_chunk | ones] [128, 65] -> accumulates A^T [64, q] in PSUM
    with the softmax denominator appearing for free in partition row 64.
  - normalize: rank-1 broadcast matmul of the denominator row -> reciprocal
    on VectorE -> multiply -> aT [64, 2048] (f32r); odd heads are DMA-shifted
    to partitions 64-127 so projection matmuls also alternate row groups.
  - proj: Y_partial[sc, jc] += aT_pair[h2].T @ W^T_h[:, jc] over 4 heads.
"""

import numpy as np

B, S, C, H = 2, 2048, 1024, 16
D = C // H            # 64
HPC = H // 4          # 4 heads per core
CS = HPC * D          # 256 channels per core
NKC = S // 128        # 16 key chunks
NSC = S // 128        # 16 row chunks
NQC = S // 512        # 4 query 512-chunks

_CACHED = {}


def _build_program():
    import concourse.bass as bass
    import concourse.tile as tile
    from concourse import bacc, mybir
    from concourse.masks import make_identity

    f32 = mybir.dt.float32
    f32r = mybir.dt.float32r

    nc = bacc.Bacc("TRN2", target_bir_lowering=False, debug=False)
    q_in = nc.dram_tensor("q_sh", [S, CS], f32, kind="ExternalInput")
    k_in = nc.dram_tensor("k_sh", [S, CS], f32, kind="ExternalInput")
    v_in = nc.dram_tensor("v_sh", [S, CS], f32, kind="ExternalInput")
    w_in = nc.dram_tensor("w_sh", [C, CS], f32, kind="ExternalInput")
    y_out = nc.dram_tensor("y_part", [S, C], f32, kind="ExternalOutput")

    with tile.TileContext(nc) as tc:
        with tc.tile_pool(name="const", bufs=1) as const_pool, \
             tc.tile_pool(name="persist", bufs=1) as persist, \
             tc.tile_pool(name="work", bufs=2) as work:

            ident = const_pool.tile([128, 128], f32)
            make_identity(nc, ident)
            ones_f32 = const_pool.tile([128, 64], f32)
            nc.vector.memset(ones_f32, 1.0)
            ones_sb = const_pool.tile([65, 64], f32r)
            nc.vector.tensor_copy(ones_sb, ones_f32[0:65, :])

            # ---- natural-layout loads ----
            v_nat = persist.tile([128, NKC, HPC, D + 1], f32r)
            for h in range(HPC):
                nc.sync.dma_start(
                    out=v_nat[:, :, h, 0:D],
                    in_=v_in[:, h * D:(h + 1) * D].rearrange(
                        "(sc p) d -> p sc d", p=128).bitcast(f32r))
            nc.vector.tensor_copy(
                v_nat[:, :, :, D:D + 1].rearrange("p s h o -> p (s h o)"),
                ones_f32[:, 0:NKC * HPC])
            w_nat = persist.tile([128, 8, CS], f32)
            nc.sync.dma_start(
                out=w_nat, in_=w_in[:, :].rearrange("(jc p) c -> p jc c", p=128))

            # ---- transposed operands ----
            # qT[h]/kT[h]: [128, S] with the [64, S] transpose duplicated on
            # both partition halves, so consecutive S^T matmuls can run on
            # alternating PE row groups (overlapped LDWEIGHTS + drains).
            qT = [persist.tile([128, S], f32r, name=f"qT{h}") for h in range(HPC)]
            kT = [persist.tile([128, S], f32r, name=f"kT{h}") for h in range(HPC)]
            # wT2: head-pair packed W^T; heads 0,2 at rows 0-63, 1,3 at 64-127
            wT2 = persist.tile([128, 2, C], f32r)

            with tc.tile_pool(name="psA", bufs=2, space="PSUM") as psA:
                for h in range(HPC):
                    for src_dram, dst in ((q_in, qT[h]), (k_in, kT[h])):
                        nat = work.tile([128, NSC, D], f32, tag="qk_nat",
                                        name="nat")
                        nc.sync.dma_start(
                            out=nat,
                            in_=src_dram[:, h * D:(h + 1) * D].rearrange(
                                "(sc p) d -> p sc d", p=128))
                        ptr = psA.tile([64, S], f32, tag="tr", name="ptr")
                        for sc in range(NSC):
                            nc.tensor.transpose(
                                ptr[:, sc * 128:(sc + 1) * 128],
                                nat[:, sc, :], ident)
                        nc.vector.tensor_copy(dst[0:64, :], ptr)
                        nc.sync.dma_start(out=dst[64:128, :], in_=dst[0:64, :])

                    # w^T for this head's channels
                    ptw = psA.tile([64, C], f32, tag="tr", name="ptw")
                    for jc in range(8):
                        nc.tensor.transpose(
                            ptw[:, jc * 128:(jc + 1) * 128],
                            w_nat[:, jc, h * D:(h + 1) * D], ident)
                    if h % 2 == 0:
                        nc.vector.tensor_copy(wT2[0:64, h // 2, :], ptw)
                    else:
                        wodd = work.tile([64, C], f32r, tag="kodd", name="wodd")
                        nc.vector.tensor_copy(wodd, ptw)
                        nc.sync.dma_start(out=wT2[64:128, h // 2, :], in_=wodd)

            # aT pairs: heads (0,1) -> aTp[0] rows 0-63/64-127, heads (2,3) -> aTp[1]
            aTp = [persist.tile([128, S], f32r, name=f"aTp{i}") for i in range(2)]

            # ---- attention per (head, query-half) ----
            # Per pass: 2 query 512-chunks. PSUM: 3 score slots [128, 1024]
            # (6 banks) + 2 AV accumulators (2 banks) = 8 banks; 3 slots give
            # cross-kc pipelining (S-matmuls of kc+1 overlap exp of kc).
            with tc.tile_pool(name="psB", bufs=1, space="PSUM") as psB:
                for h in range(HPC):
                    dst = (aTp[h // 2][0:64, :] if h % 2 == 0 else None)
                    if dst is None:
                        tmp = work.tile([64, S], f32r, tag="atmp", name="atmp")
                        dst = tmp
                    for half in range(2):
                        av = [psB.tile([65, 512], f32, tag="av", bufs=2,
                                       name=f"av{h}_{half}_{i}")
                              for i in range(2)]
                        for kc in range(NKC):
                            ps_s = psB.tile([128, 1024], f32, tag="s", bufs=3,
                                            name="ps_s")
                            for i in range(2):
                                qc = half * 2 + i
                                base = 64 * i
                                nc.tensor.matmul(
                                    ps_s[:, i * 512:(i + 1) * 512],
                                    kT[h][base:base + 64,
                                          kc * 128:(kc + 1) * 128],
                                    qT[h][base:base + 64,
                                          qc * 512:(qc + 1) * 512],
                                    start=True, stop=True)
                            exp_t = work.tile([128, 1024], f32r, tag="exp",
                                              name="exp_t")
                            nc.scalar.activation(
                                exp_t, ps_s,
                                mybir.ActivationFunctionType.Exp, scale=0.125)
                            for i in range(2):
                                nc.tensor.matmul(
                                    av[i], v_nat[:, kc, h, :],
                                    exp_t[:, i * 512:(i + 1) * 512],
                                    start=(kc == 0), stop=(kc == NKC - 1))
                        # ---- softmax normalization ----
                        for i in range(2):
                            qc = half * 2 + i
                            den = work.tile([65, 512], f32r, tag="den",
                                            name="den")
                            nc.vector.tensor_copy(den[64:65, :],
                                                  av[i][64:65, :])
                            ps_b = psB.tile([64, 512], f32, tag="s", bufs=3,
                                            name="ps_b")
                            nc.tensor.matmul(
                                ps_b, ones_sb[64:65, :], den[64:65, :],
                                start=True, stop=True)
                            rb = work.tile([64, 512], f32, tag="rb", name="rb")
                            nc.vector.reciprocal(rb, ps_b)
                            nc.vector.tensor_mul(
                                dst[:, qc * 512:(qc + 1) * 512],
                                av[i][0:64, :], rb)
                    if h % 2 == 1:
                        nc.sync.dma_start(out=aTp[h // 2][64:128, :], in_=dst)

            # ---- output projection (partial over this core's channels) ----
            with tc.tile_pool(name="psC", bufs=1, space="PSUM") as psC:
                for sc in range(NSC):
                    for jc in range(2):
                        py_a = psC.tile([128, 512], f32, tag="pya", bufs=2,
                                        name="py_a")
                        py_b = psC.tile([128, 512], f32, tag="pyb", bufs=2,
                                        name="py_b")
                        # even heads (rows 0-63) chain into py_a, odd (64-127)
                        # into py_b; adjacent matmuls alternate row groups AND
                        # banks so they overlap without bank collisions.
                        for h in range(HPC):
                            base = 64 * (h % 2)
                            nc.tensor.matmul(
                                py_a if h % 2 == 0 else py_b,
                                aTp[h // 2][base:base + 64,
                                            sc * 128:(sc + 1) * 128],
                                wT2[base:base + 64, h // 2,
                                    jc * 512:(jc + 1) * 512],
                                start=(h < 2), stop=(h >= 2))
                        y_sb = work.tile([128, 512], f32, tag="y", bufs=3,
                                         name="y_sb")
                        nc.vector.tensor_copy(y_sb, py_a)
                        nc.vector.tensor_add(y_sb, y_sb, py_b)
                        nc.sync.dma_start(
                            out=y_out[sc * 128:(sc + 1) * 128,
                                      jc * 512:(jc + 1) * 512],
                            in_=y_sb)

    nc.finalize()
    return nc


LAST_RESULT = None


def kernel(q, k, v, W_proj, attention_mask):
    """Full inputs in, full output out. attention_mask is all-ones (additive
    bias is exactly zero), so it does not need to ship to the device."""
    global LAST_RESULT
    from concourse.bass_utils import run_bass_kernel_spmd

    if "nc" not in _CACHED:
        _CACHED["nc"] = _build_program()
    nc = _CACHED["nc"]

    q = np.ascontiguousarray(np.asarray(q, dtype=np.float32))
    k = np.ascontiguousarray(np.asarray(k, dtype=np.float32))
    v = np.ascontiguousarray(np.asarray(v, dtype=np.float32))
    W = np.ascontiguousarray(np.asarray(W_proj, dtype=np.float32))

    in_maps = []
    for core in range(8):
        b, hg = divmod(core, 4)
        cs = slice(hg * CS, (hg + 1) * CS)
        in_maps.append({
            "q_sh": np.ascontiguousarray(q[b, :, cs]),
            "k_sh": np.ascontiguousarray(k[b, :, cs]),
            "v_sh": np.ascontiguousarray(v[b, :, cs]),
            "w_sh": np.ascontiguousarray(W[:, cs]),
        })

    LAST_RESULT = run_bass_kernel_spmd(nc, in_maps, core_ids=list(range(8)))
    parts = [r["y_part"] for r in LAST_RESULT.results]
    out = np.empty((B, S, C), dtype=np.float32)
    for b in range(B):
        out[b] = parts[4 * b] + parts[4 * b + 1] + parts[4 * b + 2] + parts[4 * b + 3]
    return out
